# revision 6
# baseline (speedup 1.0000x reference)
"""Causal self-attention (B=4, S=2048, D=1024, H=16, fp32) on 8 TRN2 NeuronCores.

Sharding (hybrid batch x heads): core c handles batch b = c//2 and head half
h = c%2 (8 heads, 512 channels). Each core computes:
  qkv^T slice  = (x[b] @ w_qkv[:, local])^T      via PE, transposed formulation
  per head: scores^T = K Q^T / 8, causal exp (no max subtraction -- scores are
  O(1) by construction), AV with ones-augmented V giving row sums for free,
  y^T = AV^T * (1/rowsum) broadcast via K=1 ones-matmul,
  partial out = y @ w_proj[local rows] + 0.5 * b_proj.
Host sums the two half partials per batch and stacks.

Matmul dtype is switchable: bfloat16 (fast) or float32r (tf32-class accuracy).
"""

import numpy as np
import ml_dtypes
from contextlib import ExitStack

import concourse.bass as bass
import concourse.mybir as mybir
import concourse.tile as tile
from concourse import bacc
from concourse.bass_utils import run_bass_kernel_spmd

dt = mybir.dt
AF = mybir.ActivationFunctionType

B, S, D, H, HD = 4, 2048, 1024, 16, 64
NCORES = 8
HPC = 8            # heads per core
DL = HPC * HD      # local channel width (512)
P = 128
NQG = S // 512     # q groups of 512          -> 4
NKT = S // P       # k tiles of 128           -> 16
NDC = D // P       # D chunks of 128          -> 8
NMT = 2 * DL // P  # q^T+k^T m-tiles           -> 8
NRT = S // P       # row tiles for V / proj   -> 16
GK = 1             # k-tiles per exp group

MM_DT = dt.bfloat16
NP_MM = ml_dtypes.bfloat16

DEBUG = False          # extra per-stage debug outputs (core-0 comparison)
RECIP_FAST = False     # use reciprocal_approx_fast instead of reciprocal
MASK_GPSIMD = True    # apply causal mask on GpSimd instead of VectorE

_CACHE = {}


def _emit(nc, tc, ctx, io, dbg):
    xt, wqkv, wproj, bqk, bv, bp, maskd, ones32, onesr, out_p = io

    const = ctx.enter_context(tc.tile_pool(name="const", bufs=1))
    big = ctx.enter_context(tc.tile_pool(name="big", bufs=1))
    ptp = ctx.enter_context(tc.tile_pool(name="ptp", bufs=3))
    ep = ctx.enter_context(tc.tile_pool(name="ep", bufs=3))
    outp = ctx.enter_context(tc.tile_pool(name="outp", bufs=3))
    bigps = ctx.enter_context(tc.tile_pool(name="bigps", bufs=4, space="PSUM"))
    mmps = ctx.enter_context(tc.tile_pool(name="mmps", bufs=3, space="PSUM"))
    bcps = ctx.enter_context(tc.tile_pool(name="bcps", bufs=1, space="PSUM"))

    # ---- constants / weights ----
    bqk_sb = const.tile([P, NMT], dt.float32, tag="bqk")
    nc.sync.dma_start(bqk_sb[:], bqk)
    bv_sb = const.tile([1, DL], dt.float32r, tag="bv")
    nc.sync.dma_start(bv_sb[:], bv)
    bp_sb = const.tile([1, D], dt.float32r, tag="bp")
    nc.sync.dma_start(bp_sb[:], bp)
    mask_sb = const.tile([P, 4, 512], MM_DT, tag="mask")
    nc.sync.dma_start(mask_sb[:], maskd)
    ones32_sb = const.tile([1, P], dt.float32, tag="ones32")
    nc.sync.dma_start(ones32_sb[:], ones32)
    onesr_sb = const.tile([1, P], dt.float32r, tag="onesr")
    nc.sync.dma_start(onesr_sb[:], onesr)

    xt_sb = []
    for c in range(NDC):
        t = const.tile([P, S], MM_DT, tag=f"xt{c}")
        nc.sync.dma_start(t[:], xt[c * P : (c + 1) * P, :])
        xt_sb.append(t)
    wq_sb = []
    for c in range(NDC):
        t = const.tile([P, 3 * DL], MM_DT, tag=f"wq{c}")
        nc.sync.dma_start(t[:], wqkv[c * P : (c + 1) * P, :])
        wq_sb.append(t)
    wp_sb = []
    for c in range(DL // P):
        t = const.tile([P, D], MM_DT, tag=f"wp{c}")
        nc.sync.dma_start(t[:], wproj[c * P : (c + 1) * P, :])
        wp_sb.append(t)

    # ---- persistent intermediates ----
    qkvT = const.tile([P, NMT, S], MM_DT, tag="qkvT")     # q^T then k^T
    vsb = const.tile([P, NKT, HPC, HD + 1], MM_DT, tag="vsb")
    yT = const.tile([P, DL // P, S], MM_DT, tag="yT")

    nc.gpsimd.memset(vsb[:], 1.0)  # ones column at [..., HD]

    # ---- phase 1b: V in natural layout [row, (h, hd)] ----
    def emit_v(rt):
        ps = mmps.tile([P, DL], dt.float32, tag="mm")
        for c in range(NDC):
            nc.tensor.matmul(
                ps[:],
                xt_sb[c][:, rt * P : (rt + 1) * P],
                wq_sb[c][:, 2 * DL : 3 * DL],
                start=(c == 0),
                stop=False,
            )
        nc.tensor.matmul(
            ps[:], onesr_sb[:], bv_sb[:], start=False, stop=True
        )
        nc.vector.tensor_copy(
            vsb[:, rt, :, 0:HD],
            ps[:].rearrange("p (h d) -> p h d", d=HD),
        )

    # ---- phase 1a: q^T / k^T m-tiles ----
    def qk_unit(mt, rg):
        ps = mmps.tile([P, 512], dt.float32, tag="mm", name=f"qk{mt}_{rg}")
        for c in range(NDC):
            nc.tensor.matmul(
                ps[:],
                wq_sb[c][:, mt * P : (mt + 1) * P],
                xt_sb[c][:, rg * 512 : (rg + 1) * 512],
                start=(c == 0),
                stop=(c == NDC - 1),
            )
        nc.vector.tensor_scalar_add(
            qkvT[:, mt, rg * 512 : (rg + 1) * 512],
            ps[:],
            bqk_sb[:, mt : mt + 1],
        )

    def emit_qk(mt):
        for rg in range(NQG):
            qk_unit(mt, rg)

    # ---- phase 2: attention for a head pair (concurrent K=64 row groups) ----
    def emit_pair(pr, fillers=()):
        hs = (2 * pr, 2 * pr + 1)
        fillers = list(fillers)
        fill_i = 0

        def epi(h, av, qg):
            pb = (h % 2) * 64
            if DEBUG and h == 0 and qg == 0:
                dav = ep.tile([P, 512], dt.float32, tag="dav")
                nc.vector.tensor_copy(dav[:], av[:])
                nc.sync.dma_start(dbg["dbg_av"], dav[:])
            yraw = ep.tile([HD + 1, 512], dt.float32, tag="yraw")
            nc.vector.tensor_copy(yraw[:], av[0 : HD + 1, :])
            lnd = ep.tile([1, 512], dt.float32, tag="lnd")
            nc.scalar.activation(lnd[:], yraw[HD : HD + 1, :], AF.Ln)
            rcp = ep.tile([1, 512], dt.float32, tag="rcp")
            nc.scalar.activation(rcp[:], lnd[:], AF.Exp, scale=-1.0)
            bc = bcps.tile([64, 512], dt.float32, tag="bc")
            nc.tensor.matmul(bc[:], ones32_sb[:, 0:64], rcp[:], start=True, stop=True)
            nc.vector.tensor_mul(
                yT[pb : pb + 64, h // 2, qg * 512 : (qg + 1) * 512],
                yraw[0:HD, :],
                bc[:],
            )

        nfill_slots = sum(4 * qg + 4 for qg in range(NQG))
        fill_every = max(1, nfill_slots // max(1, len(fillers))) if fillers else 0
        slot = 0
        for qg in range(NQG):
            nkt = 4 * qg + 4
            av = {h: mmps.tile([P, 512], dt.float32, tag="mm", name=f"av{h}") for h in hs}
            for kt in range(nkt):
                sps = {h: bigps.tile([P, 512], dt.float32, tag="big", name=f"sps{h}") for h in hs}
                with tc.tile_critical():
                    for h in hs:
                        pb = (h % 2) * 64
                        nc.tensor.matmul(
                            sps[h][:],
                            qkvT[pb : pb + 64, DL // P + h // 2, kt * P : (kt + 1) * P],
                            qkvT[pb : pb + 64, h // 2, qg * 512 : (qg + 1) * 512],
                            start=True,
                            stop=True,
                        )
                pt = {}
                td = kt - 4 * qg
                for h in hs:
                    pt[h] = ptp.tile([P, 512], MM_DT, tag="pt", name=f"pt{h}")
                    nc.scalar.activation(pt[h][:], sps[h][:], AF.Exp, scale=0.125)
                    if td >= 0:
                        eng = nc.gpsimd if MASK_GPSIMD else nc.vector
                        eng.tensor_tensor(
                            pt[h][:],
                            pt[h][:],
                            mask_sb[:, td, :],
                            mybir.AluOpType.mult,
                        )
                if DEBUG and pr == 0 and qg == 0 and kt == 0:
                    dpt = ep.tile([P, 512], dt.float32, tag="dpt")
                    nc.vector.tensor_copy(dpt[:], pt[0][:])
                    nc.sync.dma_start(dbg["dbg_pt"], dpt[:])
                for h in hs:
                    nc.tensor.matmul(
                        av[h][0 : HD + 1, :],
                        vsb[:, kt, h, :],
                        pt[h][:],
                        start=(kt == 0),
                        stop=(kt == nkt - 1),
                    )
                slot += 1
                if fillers and fill_i < len(fillers) and slot % fill_every == 0:
                    fillers[fill_i]()
                    fill_i += 1
            for h in hs:
                epi(h, av[h], qg)
        while fill_i < len(fillers):
            fillers[fill_i]()
            fill_i += 1

    # ---- phase 3: projection ----
    def emit_proj(qt):
        for ng in range(D // 512):
            ps = mmps.tile([P, 512], dt.float32, tag="mm")
            for c in range(DL // P):
                nc.tensor.matmul(
                    ps[:],
                    yT[:, c, qt * P : (qt + 1) * P],
                    wp_sb[c][:, ng * 512 : (ng + 1) * 512],
                    start=(c == 0),
                    stop=False,
                )
            nc.tensor.matmul(
                ps[:], onesr_sb[:], bp_sb[:, ng * 512 : (ng + 1) * 512],
                start=False, stop=True,
            )
            o = outp.tile([P, 512], dt.float32, tag="o")
            nc.vector.tensor_copy(o[:], ps[:])
            nc.sync.dma_start(out_p[qt * P : (qt + 1) * P, ng * 512 : (ng + 1) * 512], o[:])

    # emission order: early V tiles + first pair's QK, then attention with
    # next pair's QK units interleaved as PE filler
    for rt in range(4):
        emit_v(rt)
    emit_qk(0)
    emit_qk(DL // P)
    for rt in range(4, NRT):
        emit_v(rt)
    for pair in range(HPC // 2):
        if pair + 1 < HPC // 2:
            fill = [
                (lambda mt=mt, rg=rg: qk_unit(mt, rg))
                for mt in (pair + 1, DL // P + pair + 1)
                for rg in range(NQG)
            ]
        else:
            fill = []
        emit_pair(pair, fill)
    if DEBUG:
        nc.sync.dma_start(dbg["dbg_qkvT"], qkvT[:].rearrange("p a b -> p (a b)"))
        nc.sync.dma_start(dbg["dbg_vsb"], vsb[:].rearrange("p a b c -> p (a b c)"))
        nc.sync.dma_start(dbg["dbg_yT"], yT[:].rearrange("p a b -> p (a b)"))
    for qt in range(NRT):
        emit_proj(qt)


def _build():
    if "nc" in _CACHE:
        return _CACHE["nc"]
    nc = bacc.Bacc("TRN2", target_bir_lowering=False, debug=False, num_devices=NCORES)
    xt = nc.dram_tensor("xt", [D, S], MM_DT, kind="ExternalInput").ap()
    wqkv = nc.dram_tensor("wqkv", [D, 3 * DL], MM_DT, kind="ExternalInput").ap()
    wproj = nc.dram_tensor("wproj", [DL, D], MM_DT, kind="ExternalInput").ap()
    bqk = nc.dram_tensor("bqk", [P, NMT], dt.float32, kind="ExternalInput").ap()
    bv = nc.dram_tensor("bv", [1, DL], dt.float32r, kind="ExternalInput").ap()
    bp = nc.dram_tensor("bp", [1, D], dt.float32r, kind="ExternalInput").ap()
    maskd = nc.dram_tensor("maskd", [P, 4, 512], MM_DT, kind="ExternalInput").ap()
    ones32 = nc.dram_tensor("ones32", [1, P], dt.float32, kind="ExternalInput").ap()
    onesr = nc.dram_tensor("onesr", [1, P], dt.float32r, kind="ExternalInput").ap()
    out_p = nc.dram_tensor("out_p", [S, D], dt.float32, kind="ExternalOutput").ap()

    dbg = {}
    if DEBUG:
        dbg["dbg_qkvT"] = nc.dram_tensor("dbg_qkvT", [P, NMT * S], MM_DT, kind="ExternalOutput").ap()
        dbg["dbg_vsb"] = nc.dram_tensor("dbg_vsb", [P, NKT * HPC * (HD + 1)], MM_DT, kind="ExternalOutput").ap()
        dbg["dbg_yT"] = nc.dram_tensor("dbg_yT", [P, (DL // P) * S], MM_DT, kind="ExternalOutput").ap()
        dbg["dbg_pt"] = nc.dram_tensor("dbg_pt", [P, GK * 512], dt.float32, kind="ExternalOutput").ap()
        dbg["dbg_av"] = nc.dram_tensor("dbg_av", [P, 512], dt.float32, kind="ExternalOutput").ap()

    io = (xt, wqkv, wproj, bqk, bv, bp, maskd, ones32, onesr, out_p)
    with tile.TileContext(nc) as tc, ExitStack() as ctx:
        _emit(nc, tc, ctx, io, dbg)
    nc.compile()
    _CACHE["nc"] = nc
    return nc


def _in_maps(x, w_qkv, b_qkv, w_proj, b_proj):
    x = np.asarray(x, dtype=np.float32)
    w_qkv = np.asarray(w_qkv, dtype=np.float32)
    b_qkv = np.asarray(b_qkv, dtype=np.float32)
    w_proj = np.asarray(w_proj, dtype=np.float32)
    b_proj = np.asarray(b_proj, dtype=np.float32)

    # causal mask for the 4 diagonal-tile alignments: [128, 4, 512]
    kp = np.arange(P)[:, None, None]
    td = np.arange(4)[None, :, None]
    qf = np.arange(512)[None, None, :]
    maskd = ((P * td + kp) <= qf).astype(NP_MM)
    ones = np.ones((1, P), dtype=np.float32)

    maps = []
    for c in range(NCORES):
        b, half = divmod(c, 2)
        lo, hi = half * DL, (half + 1) * DL
        wq = w_qkv[:, lo:hi]
        wk = w_qkv[:, D + lo : D + hi]
        wv = w_qkv[:, 2 * D + lo : 2 * D + hi]
        wqkv_l = np.concatenate([wq, wk, wv], axis=1).astype(NP_MM)
        bqk_l = np.concatenate([b_qkv[lo:hi], b_qkv[D + lo : D + hi]])
        bqk_t = np.ascontiguousarray(bqk_l.reshape(NMT, P).T)  # [128, 8]
        maps.append(
            {
                "xt": np.ascontiguousarray(x[b].T).astype(NP_MM),
                "wqkv": wqkv_l,
                "wproj": w_proj[lo:hi, :].astype(NP_MM),
                "bqk": bqk_t,
                "bv": b_qkv[2 * D + lo : 2 * D + hi].reshape(1, DL),
                "bp": (0.5 * b_proj).reshape(1, D),
                "maskd": maskd,
                "ones32": ones,
                "onesr": ones,
            }
        )
    return maps


def _run(x, w_qkv, b_qkv, w_proj, b_proj, trace=False):
    nc = _build()
    maps = _in_maps(x, w_qkv, b_qkv, w_proj, b_proj)
    res = run_bass_kernel_spmd(nc, maps, list(range(NCORES)), trace=trace)
    out = np.empty((B, S, D), dtype=np.float32)
    for b in range(B):
        out[b] = res.results[2 * b]["out_p"] + res.results[2 * b + 1]["out_p"]
    return out, res


def kernel(x, w_qkv, b_qkv, w_proj, b_proj):
    out, _ = _run(x, w_qkv, b_qkv, w_proj, b_proj)
    return out


# revision 9
# speedup vs baseline: 1.2250x; 1.2250x over previous
"""Causal self-attention (B=4, S=2048, D=1024, H=16, fp32) on 8 TRN2 NeuronCores.

Sharding (hybrid batch x heads): core c handles batch b = c//2 and head half
h = c%2 (8 heads, 512 channels). Each core computes:
  qkv^T slice  = (x[b] @ w_qkv[:, local])^T      via PE, transposed formulation
  per head: scores^T = K Q^T / 8, causal exp (no max subtraction -- scores are
  O(1) by construction), AV with ones-augmented V giving row sums for free,
  y^T = AV^T * (1/rowsum) broadcast via K=1 ones-matmul,
  partial out = y @ w_proj[local rows] + 0.5 * b_proj.
Host sums the two half partials per batch and stacks.

Matmul dtype is switchable: bfloat16 (fast) or float32r (tf32-class accuracy).
"""

import numpy as np
import ml_dtypes
from contextlib import ExitStack

import concourse.bass as bass
import concourse.mybir as mybir
import concourse.tile as tile
from concourse import bacc
from concourse.bass_utils import run_bass_kernel_spmd

dt = mybir.dt
AF = mybir.ActivationFunctionType

B, S, D, H, HD = 4, 2048, 1024, 16, 64
NCORES = 8
HPC = 8            # heads per core
DL = HPC * HD      # local channel width (512)
P = 128
NQG = S // 512     # q groups of 512          -> 4
NKT = S // P       # k tiles of 128           -> 16
NDC = D // P       # D chunks of 128          -> 8
NMT = 2 * DL // P  # q^T+k^T m-tiles           -> 8
NRT = S // P       # row tiles for V / proj   -> 16
GK = 1             # k-tiles per exp group

MM_DT = dt.bfloat16
NP_MM = ml_dtypes.bfloat16

DEBUG = False          # extra per-stage debug outputs (core-0 comparison)
RECIP_FAST = False     # use reciprocal_approx_fast instead of reciprocal
MASK_GPSIMD = False    # apply causal mask on GpSimd instead of VectorE

_CACHE = {}


def _emit(nc, tc, ctx, io, dbg):
    xt, wqkv, wproj, bqk, bv, bp, maskd, ones32, onesr, out_p = io

    const = ctx.enter_context(tc.tile_pool(name="const", bufs=1))
    big = ctx.enter_context(tc.tile_pool(name="big", bufs=1))
    ptp = ctx.enter_context(tc.tile_pool(name="ptp", bufs=5))
    ep = ctx.enter_context(tc.tile_pool(name="ep", bufs=3))
    outp = ctx.enter_context(tc.tile_pool(name="outp", bufs=3))
    bigps = ctx.enter_context(tc.tile_pool(name="bigps", bufs=4, space="PSUM"))
    mmps = ctx.enter_context(tc.tile_pool(name="mmps", bufs=3, space="PSUM"))
    bcps = ctx.enter_context(tc.tile_pool(name="bcps", bufs=1, space="PSUM"))

    # ---- constants / weights ----
    bqk_sb = const.tile([P, NMT], dt.float32, tag="bqk")
    nc.sync.dma_start(bqk_sb[:], bqk)
    bv_sb = const.tile([1, DL], dt.float32r, tag="bv")
    nc.sync.dma_start(bv_sb[:], bv)
    bp_sb = const.tile([1, D], dt.float32r, tag="bp")
    nc.sync.dma_start(bp_sb[:], bp)
    mask_sb = const.tile([P, 4, 512], MM_DT, tag="mask")
    nc.sync.dma_start(mask_sb[:], maskd)
    ones32_sb = const.tile([1, P], dt.float32, tag="ones32")
    nc.sync.dma_start(ones32_sb[:], ones32)
    onesr_sb = const.tile([1, P], dt.float32r, tag="onesr")
    nc.sync.dma_start(onesr_sb[:], onesr)
    ones512_sb = const.tile([HD + 1, 512], dt.float32, tag="ones512")
    nc.vector.memset(ones512_sb[:], 1.0)

    xt_sb = []
    for c in range(NDC):
        t = const.tile([P, S], MM_DT, tag=f"xt{c}")
        nc.sync.dma_start(t[:], xt[c * P : (c + 1) * P, :])
        xt_sb.append(t)
    wq_sb = []
    for c in range(NDC):
        t = const.tile([P, 3 * DL], MM_DT, tag=f"wq{c}")
        nc.sync.dma_start(t[:], wqkv[c * P : (c + 1) * P, :])
        wq_sb.append(t)
    wp_sb = []
    for c in range(DL // P):
        t = const.tile([P, D], MM_DT, tag=f"wp{c}")
        nc.sync.dma_start(t[:], wproj[c * P : (c + 1) * P, :])
        wp_sb.append(t)

    # ---- persistent intermediates ----
    qkvT = const.tile([P, NMT, S], MM_DT, tag="qkvT")     # q^T then k^T
    vsb = const.tile([P, NKT, HPC, HD + 1], MM_DT, tag="vsb")
    yT = const.tile([P, DL // P, S], MM_DT, tag="yT")

    nc.gpsimd.memset(vsb[:], 1.0)  # ones column at [..., HD]

    # ---- phase 1b: V in natural layout [row, (h, hd)] ----
    def emit_v(rt):
        ps = mmps.tile([P, DL], dt.float32, tag="mm")
        for c in range(NDC):
            nc.tensor.matmul(
                ps[:],
                xt_sb[c][:, rt * P : (rt + 1) * P],
                wq_sb[c][:, 2 * DL : 3 * DL],
                start=(c == 0),
                stop=False,
            )
        nc.tensor.matmul(
            ps[:], onesr_sb[:], bv_sb[:], start=False, stop=True
        )
        nc.vector.tensor_copy(
            vsb[:, rt, :, 0:HD],
            ps[:].rearrange("p (h d) -> p h d", d=HD),
        )

    # ---- phase 1a: q^T / k^T m-tiles ----
    def qk_unit(mt, rg):
        ps = mmps.tile([P, 512], dt.float32, tag="mm", name=f"qk{mt}_{rg}")
        for c in range(NDC):
            nc.tensor.matmul(
                ps[:],
                wq_sb[c][:, mt * P : (mt + 1) * P],
                xt_sb[c][:, rg * 512 : (rg + 1) * 512],
                start=(c == 0),
                stop=(c == NDC - 1),
            )
        nc.scalar.activation(
            qkvT[:, mt, rg * 512 : (rg + 1) * 512],
            ps[:],
            AF.Identity,
            bias=bqk_sb[:, mt : mt + 1],
        )

    def emit_qk(mt):
        for rg in range(NQG):
            qk_unit(mt, rg)

    # ---- phase 2: attention for a head pair (concurrent K=64 row groups) ----
    def emit_pair(pr, fillers=()):
        hs = (2 * pr, 2 * pr + 1)
        fillers = list(fillers)
        fill_i = 0

        def epi(h, av, qg):
            pb = (h % 2) * 64
            if DEBUG and h == 0 and qg == 0:
                dav = ep.tile([P, 512], dt.float32, tag="dav")
                nc.vector.tensor_copy(dav[:], av[:])
                nc.sync.dma_start(dbg["dbg_av"], dav[:])
            yraw = ep.tile([HD + 1, 512], dt.float32, tag="yraw")
            nc.vector.tensor_copy(yraw[:], av[0 : HD + 1, :])
            rcp = ep.tile([1, 512], dt.float32, tag="rcp")
            nc.vector.reciprocal(rcp[:], yraw[HD : HD + 1, :])
            bc = bcps.tile([64, 512], dt.float32, tag="bc")
            nc.tensor.matmul(bc[:], ones32_sb[:, 0:64], rcp[:], start=True, stop=True)
            nc.vector.tensor_mul(
                yT[pb : pb + 64, h // 2, qg * 512 : (qg + 1) * 512],
                yraw[0:HD, :],
                bc[:],
            )

        nfill_slots = sum(4 * qg + 4 for qg in range(NQG))
        fill_every = max(1, nfill_slots // max(1, len(fillers))) if fillers else 0
        slot = 0
        for qg in range(NQG):
            nkt = 4 * qg + 4
            av = {h: mmps.tile([P, 512], dt.float32, tag="mm", name=f"av{h}") for h in hs}
            for kt in range(nkt):
                sps = {h: bigps.tile([P, 512], dt.float32, tag="big", name=f"sps{h}") for h in hs}
                for h in hs:
                    pb = (h % 2) * 64
                    nc.tensor.matmul(
                        sps[h][:],
                        qkvT[pb : pb + 64, DL // P + h // 2, kt * P : (kt + 1) * P],
                        qkvT[pb : pb + 64, h // 2, qg * 512 : (qg + 1) * 512],
                        start=True,
                        stop=True,
                    )
                pt = {}
                td = kt - 4 * qg
                for h in hs:
                    pt[h] = ptp.tile([P, 512], MM_DT, tag="pt", name=f"pt{h}")
                    nc.scalar.activation(pt[h][:], sps[h][:], AF.Exp, scale=0.125)
                    if td >= 0:
                        eng = nc.gpsimd if MASK_GPSIMD else nc.vector
                        eng.tensor_tensor(
                            pt[h][:],
                            pt[h][:],
                            mask_sb[:, td, :],
                            mybir.AluOpType.mult,
                        )
                if DEBUG and pr == 0 and qg == 0 and kt == 0:
                    dpt = ep.tile([P, 512], dt.float32, tag="dpt")
                    nc.vector.tensor_copy(dpt[:], pt[0][:])
                    nc.sync.dma_start(dbg["dbg_pt"], dpt[:])
                for h in hs:
                    nc.tensor.matmul(
                        av[h][0 : HD + 1, :],
                        vsb[:, kt, h, :],
                        pt[h][:],
                        start=(kt == 0),
                        stop=(kt == nkt - 1),
                    )
                slot += 1
                if fillers and fill_i < len(fillers) and slot % fill_every == 0:
                    fillers[fill_i]()
                    fill_i += 1
            for h in hs:
                epi(h, av[h], qg)
        while fill_i < len(fillers):
            fillers[fill_i]()
            fill_i += 1

    # ---- phase 3: projection ----
    def emit_proj(qt):
        for ng in range(D // 512):
            ps = mmps.tile([P, 512], dt.float32, tag="mm")
            for c in range(DL // P):
                nc.tensor.matmul(
                    ps[:],
                    yT[:, c, qt * P : (qt + 1) * P],
                    wp_sb[c][:, ng * 512 : (ng + 1) * 512],
                    start=(c == 0),
                    stop=False,
                )
            nc.tensor.matmul(
                ps[:], onesr_sb[:], bp_sb[:, ng * 512 : (ng + 1) * 512],
                start=False, stop=True,
            )
            o = outp.tile([P, 512], dt.float32, tag="o")
            nc.scalar.activation(o[:], ps[:], AF.Copy)
            nc.sync.dma_start(out_p[qt * P : (qt + 1) * P, ng * 512 : (ng + 1) * 512], o[:])

    # emission order: early V tiles + first pair's QK, then attention with
    # next pair's QK units interleaved as PE filler
    for rt in range(4):
        emit_v(rt)
    emit_qk(0)
    emit_qk(DL // P)
    for rt in range(4, NRT):
        emit_v(rt)
    for pair in range(HPC // 2):
        if pair + 1 < HPC // 2:
            fill = [
                (lambda mt=mt, rg=rg: qk_unit(mt, rg))
                for mt in (pair + 1, DL // P + pair + 1)
                for rg in range(NQG)
            ]
        else:
            fill = []
        emit_pair(pair, fill)
    if DEBUG:
        nc.sync.dma_start(dbg["dbg_qkvT"], qkvT[:].rearrange("p a b -> p (a b)"))
        nc.sync.dma_start(dbg["dbg_vsb"], vsb[:].rearrange("p a b c -> p (a b c)"))
        nc.sync.dma_start(dbg["dbg_yT"], yT[:].rearrange("p a b -> p (a b)"))
    for qt in range(NRT):
        emit_proj(qt)


def _build():
    if "nc" in _CACHE:
        return _CACHE["nc"]
    nc = bacc.Bacc("TRN2", target_bir_lowering=False, debug=False, num_devices=NCORES)
    xt = nc.dram_tensor("xt", [D, S], MM_DT, kind="ExternalInput").ap()
    wqkv = nc.dram_tensor("wqkv", [D, 3 * DL], MM_DT, kind="ExternalInput").ap()
    wproj = nc.dram_tensor("wproj", [DL, D], MM_DT, kind="ExternalInput").ap()
    bqk = nc.dram_tensor("bqk", [P, NMT], dt.float32, kind="ExternalInput").ap()
    bv = nc.dram_tensor("bv", [1, DL], dt.float32r, kind="ExternalInput").ap()
    bp = nc.dram_tensor("bp", [1, D], dt.float32r, kind="ExternalInput").ap()
    maskd = nc.dram_tensor("maskd", [P, 4, 512], MM_DT, kind="ExternalInput").ap()
    ones32 = nc.dram_tensor("ones32", [1, P], dt.float32, kind="ExternalInput").ap()
    onesr = nc.dram_tensor("onesr", [1, P], dt.float32r, kind="ExternalInput").ap()
    out_p = nc.dram_tensor("out_p", [S, D], dt.float32, kind="ExternalOutput").ap()

    dbg = {}
    if DEBUG:
        dbg["dbg_qkvT"] = nc.dram_tensor("dbg_qkvT", [P, NMT * S], MM_DT, kind="ExternalOutput").ap()
        dbg["dbg_vsb"] = nc.dram_tensor("dbg_vsb", [P, NKT * HPC * (HD + 1)], MM_DT, kind="ExternalOutput").ap()
        dbg["dbg_yT"] = nc.dram_tensor("dbg_yT", [P, (DL // P) * S], MM_DT, kind="ExternalOutput").ap()
        dbg["dbg_pt"] = nc.dram_tensor("dbg_pt", [P, GK * 512], dt.float32, kind="ExternalOutput").ap()
        dbg["dbg_av"] = nc.dram_tensor("dbg_av", [P, 512], dt.float32, kind="ExternalOutput").ap()

    io = (xt, wqkv, wproj, bqk, bv, bp, maskd, ones32, onesr, out_p)
    with tile.TileContext(nc) as tc, ExitStack() as ctx:
        _emit(nc, tc, ctx, io, dbg)
    nc.compile()
    _CACHE["nc"] = nc
    return nc


def _in_maps(x, w_qkv, b_qkv, w_proj, b_proj):
    x = np.asarray(x, dtype=np.float32)
    w_qkv = np.asarray(w_qkv, dtype=np.float32)
    b_qkv = np.asarray(b_qkv, dtype=np.float32)
    w_proj = np.asarray(w_proj, dtype=np.float32)
    b_proj = np.asarray(b_proj, dtype=np.float32)

    # causal mask for the 4 diagonal-tile alignments: [128, 4, 512]
    kp = np.arange(P)[:, None, None]
    td = np.arange(4)[None, :, None]
    qf = np.arange(512)[None, None, :]
    maskd = ((P * td + kp) <= qf).astype(NP_MM)
    ones = np.ones((1, P), dtype=np.float32)

    maps = []
    for c in range(NCORES):
        b, half = divmod(c, 2)
        lo, hi = half * DL, (half + 1) * DL
        wq = w_qkv[:, lo:hi]
        wk = w_qkv[:, D + lo : D + hi]
        wv = w_qkv[:, 2 * D + lo : 2 * D + hi]
        wqkv_l = np.concatenate([wq, wk, wv], axis=1).astype(NP_MM)
        bqk_l = np.concatenate([b_qkv[lo:hi], b_qkv[D + lo : D + hi]])
        bqk_t = np.ascontiguousarray(bqk_l.reshape(NMT, P).T)  # [128, 8]
        maps.append(
            {
                "xt": np.ascontiguousarray(x[b].T).astype(NP_MM),
                "wqkv": wqkv_l,
                "wproj": w_proj[lo:hi, :].astype(NP_MM),
                "bqk": bqk_t,
                "bv": b_qkv[2 * D + lo : 2 * D + hi].reshape(1, DL),
                "bp": (0.5 * b_proj).reshape(1, D),
                "maskd": maskd,
                "ones32": ones,
                "onesr": ones,
            }
        )
    return maps


def _run(x, w_qkv, b_qkv, w_proj, b_proj, trace=False):
    nc = _build()
    maps = _in_maps(x, w_qkv, b_qkv, w_proj, b_proj)
    res = run_bass_kernel_spmd(nc, maps, list(range(NCORES)), trace=trace)
    out = np.empty((B, S, D), dtype=np.float32)
    for b in range(B):
        out[b] = res.results[2 * b]["out_p"] + res.results[2 * b + 1]["out_p"]
    return out, res


def kernel(x, w_qkv, b_qkv, w_proj, b_proj):
    out, _ = _run(x, w_qkv, b_qkv, w_proj, b_proj)
    return out


# revision 11
# speedup vs baseline: 1.2568x; 1.0260x over previous
"""Causal self-attention (B=4, S=2048, D=1024, H=16, fp32) on 8 TRN2 NeuronCores.

Sharding (hybrid batch x heads): core c handles batch b = c//2 and head half
h = c%2 (8 heads, 512 channels). Each core computes:
  qkv^T slice  = (x[b] @ w_qkv[:, local])^T      via PE, transposed formulation
  per head: scores^T = K Q^T / 8, causal exp (no max subtraction -- scores are
  O(1) by construction), AV with ones-augmented V giving row sums for free,
  y^T = AV^T * (1/rowsum) broadcast via K=1 ones-matmul,
  partial out = y @ w_proj[local rows] + 0.5 * b_proj.
Host sums the two half partials per batch and stacks.

Matmul dtype is switchable: bfloat16 (fast) or float32r (tf32-class accuracy).
"""

import numpy as np
import ml_dtypes
from contextlib import ExitStack

import concourse.bass as bass
import concourse.mybir as mybir
import concourse.tile as tile
from concourse import bacc
from concourse.bass_utils import run_bass_kernel_spmd

dt = mybir.dt
AF = mybir.ActivationFunctionType

B, S, D, H, HD = 4, 2048, 1024, 16, 64
NCORES = 8
HPC = 8            # heads per core
DL = HPC * HD      # local channel width (512)
P = 128
NQG = S // 512     # q groups of 512          -> 4
NKT = S // P       # k tiles of 128           -> 16
NDC = D // P       # D chunks of 128          -> 8
NMT = 2 * DL // P  # q^T+k^T m-tiles           -> 8
NRT = S // P       # row tiles for V / proj   -> 16
GK = 1             # k-tiles per exp group

MM_DT = dt.bfloat16
NP_MM = ml_dtypes.bfloat16

DEBUG = False          # extra per-stage debug outputs (core-0 comparison)
RECIP_FAST = False     # use reciprocal_approx_fast instead of reciprocal
MASK_GPSIMD = False    # apply causal mask on GpSimd instead of VectorE

_CACHE = {}


def _emit(nc, tc, ctx, io, dbg):
    xt, wqkv, wproj, bqk, bv, bp, maskd, ones32, onesr, out_p = io

    const = ctx.enter_context(tc.tile_pool(name="const", bufs=1))
    big = ctx.enter_context(tc.tile_pool(name="big", bufs=1))
    ptp = ctx.enter_context(tc.tile_pool(name="ptp", bufs=6))
    ep = ctx.enter_context(tc.tile_pool(name="ep", bufs=3))
    outp = ctx.enter_context(tc.tile_pool(name="outp", bufs=3))
    bigps = ctx.enter_context(tc.tile_pool(name="bigps", bufs=4, space="PSUM"))
    mmps = ctx.enter_context(tc.tile_pool(name="mmps", bufs=2, space="PSUM"))
    avps = ctx.enter_context(tc.tile_pool(name="avps", bufs=2, space="PSUM"))

    # ---- constants / weights ----
    bqk_sb = const.tile([P, NMT], dt.float32, tag="bqk")
    nc.sync.dma_start(bqk_sb[:], bqk)
    bv_sb = const.tile([1, DL], dt.float32r, tag="bv")
    nc.sync.dma_start(bv_sb[:], bv)
    bp_sb = const.tile([1, D], dt.float32r, tag="bp")
    nc.sync.dma_start(bp_sb[:], bp)
    mask_sb = const.tile([P, 4, 512], MM_DT, tag="mask")
    nc.sync.dma_start(mask_sb[:], maskd)
    ones32_sb = const.tile([1, P], dt.float32, tag="ones32")
    nc.sync.dma_start(ones32_sb[:], ones32)
    onesr_sb = const.tile([1, P], dt.float32r, tag="onesr")
    nc.sync.dma_start(onesr_sb[:], onesr)
    ones512_sb = const.tile([HD + 1, 512], dt.float32, tag="ones512")
    nc.vector.memset(ones512_sb[:], 1.0)

    xt_sb = []
    for c in range(NDC):
        t = const.tile([P, S], MM_DT, tag=f"xt{c}")
        nc.sync.dma_start(t[:], xt[c * P : (c + 1) * P, :])
        xt_sb.append(t)
    wq_sb = []
    for c in range(NDC):
        t = const.tile([P, 3 * DL], MM_DT, tag=f"wq{c}")
        nc.sync.dma_start(t[:], wqkv[c * P : (c + 1) * P, :])
        wq_sb.append(t)
    wp_sb = []
    for c in range(DL // P):
        t = const.tile([P, D], MM_DT, tag=f"wp{c}")
        nc.sync.dma_start(t[:], wproj[c * P : (c + 1) * P, :])
        wp_sb.append(t)

    # ---- persistent intermediates ----
    qkvT = const.tile([P, NMT, S], MM_DT, tag="qkvT")     # q^T then k^T
    vsb = const.tile([P, NKT, HPC, HD + 1], MM_DT, tag="vsb")
    yT = const.tile([P, DL // P, S], MM_DT, tag="yT")

    nc.gpsimd.memset(vsb[:], 1.0)  # ones column at [..., HD]

    # ---- phase 1b: V in natural layout [row, (h, hd)] ----
    def emit_v(rt):
        ps = mmps.tile([P, DL], dt.float32, tag="mm")
        for c in range(NDC):
            nc.tensor.matmul(
                ps[:],
                xt_sb[c][:, rt * P : (rt + 1) * P],
                wq_sb[c][:, 2 * DL : 3 * DL],
                start=(c == 0),
                stop=False,
            )
        nc.tensor.matmul(
            ps[:], onesr_sb[:], bv_sb[:], start=False, stop=True
        )
        nc.vector.tensor_copy(
            vsb[:, rt, :, 0:HD],
            ps[:].rearrange("p (h d) -> p h d", d=HD),
        )

    # ---- phase 1a: q^T / k^T m-tiles ----
    def qk_unit(mt, rg):
        ps = mmps.tile([P, 512], dt.float32, tag="mm", name=f"qk{mt}_{rg}")
        for c in range(NDC):
            nc.tensor.matmul(
                ps[:],
                wq_sb[c][:, mt * P : (mt + 1) * P],
                xt_sb[c][:, rg * 512 : (rg + 1) * 512],
                start=(c == 0),
                stop=(c == NDC - 1),
            )
        nc.scalar.activation(
            qkvT[:, mt, rg * 512 : (rg + 1) * 512],
            ps[:],
            AF.Identity,
            bias=bqk_sb[:, mt : mt + 1],
        )

    def emit_qk(mt):
        for rg in range(NQG):
            qk_unit(mt, rg)

    # ---- phase 2: attention for a head pair (concurrent K=64 row groups) ----
    def emit_pair(pr, fillers=()):
        hs = (2 * pr, 2 * pr + 1)
        fillers = list(fillers)
        fill_i = 0

        def epi(h, av, qg):
            pb = (h % 2) * 64
            if DEBUG and h == 0 and qg == 0:
                dav = ep.tile([P, 512], dt.float32, tag="dav")
                nc.vector.tensor_copy(dav[:], av[:])
                nc.sync.dma_start(dbg["dbg_av"], dav[:])
            yraw = ep.tile([HD + 1, 512], dt.float32, tag="yraw")
            nc.vector.tensor_copy(yraw[:], av[0 : HD + 1, :])
            rcp = ep.tile([1, 512], dt.float32r, tag="rcp")
            with nc.allow_low_precision(reason="softmax denominators ~1e3; f32r err ~1e-4"):
                nc.vector.reciprocal(rcp[:], yraw[HD : HD + 1, :])
            bc = mmps.tile([64, 512], dt.float32, tag="mm", name="bc")
            nc.tensor.matmul(bc[:], onesr_sb[:, 0:64], rcp[:], start=True, stop=True)
            nc.vector.tensor_mul(
                yT[pb : pb + 64, h // 2, qg * 512 : (qg + 1) * 512],
                yraw[0:HD, :],
                bc[:],
            )

        nfill_slots = sum(4 * qg + 4 for qg in range(NQG))
        fill_every = max(1, nfill_slots // max(1, len(fillers))) if fillers else 0
        slot = 0
        for qg in range(NQG):
            nkt = 4 * qg + 4
            av = {h: avps.tile([P, 512], dt.float32, tag="av", name=f"av{h}") for h in hs}
            for kt in range(nkt):
                sps = {h: bigps.tile([P, 512], dt.float32, tag="big", name=f"sps{h}") for h in hs}
                for h in hs:
                    pb = (h % 2) * 64
                    nc.tensor.matmul(
                        sps[h][:],
                        qkvT[pb : pb + 64, DL // P + h // 2, kt * P : (kt + 1) * P],
                        qkvT[pb : pb + 64, h // 2, qg * 512 : (qg + 1) * 512],
                        start=True,
                        stop=True,
                    )
                pt = {}
                td = kt - 4 * qg
                for h in hs:
                    pt[h] = ptp.tile([P, 512], MM_DT, tag="pt", name=f"pt{h}")
                    nc.scalar.activation(pt[h][:], sps[h][:], AF.Exp, scale=0.125)
                    if td >= 0:
                        eng = nc.gpsimd if MASK_GPSIMD else nc.vector
                        eng.tensor_tensor(
                            pt[h][:],
                            pt[h][:],
                            mask_sb[:, td, :],
                            mybir.AluOpType.mult,
                        )
                if DEBUG and pr == 0 and qg == 0 and kt == 0:
                    dpt = ep.tile([P, 512], dt.float32, tag="dpt")
                    nc.vector.tensor_copy(dpt[:], pt[0][:])
                    nc.sync.dma_start(dbg["dbg_pt"], dpt[:])
                for h in hs:
                    nc.tensor.matmul(
                        av[h][0 : HD + 1, :],
                        vsb[:, kt, h, :],
                        pt[h][:],
                        start=(kt == 0),
                        stop=(kt == nkt - 1),
                    )
                slot += 1
                if fillers and fill_i < len(fillers) and slot % fill_every == 0:
                    fillers[fill_i]()
                    fill_i += 1
            for h in hs:
                epi(h, av[h], qg)
        while fill_i < len(fillers):
            fillers[fill_i]()
            fill_i += 1

    # ---- phase 3: projection ----
    def emit_proj(qt):
        for ng in range(D // 512):
            ps = mmps.tile([P, 512], dt.float32, tag="mm")
            for c in range(DL // P):
                nc.tensor.matmul(
                    ps[:],
                    yT[:, c, qt * P : (qt + 1) * P],
                    wp_sb[c][:, ng * 512 : (ng + 1) * 512],
                    start=(c == 0),
                    stop=False,
                )
            nc.tensor.matmul(
                ps[:], onesr_sb[:], bp_sb[:, ng * 512 : (ng + 1) * 512],
                start=False, stop=True,
            )
            o = outp.tile([P, 512], dt.float32, tag="o")
            nc.scalar.activation(o[:], ps[:], AF.Copy)
            nc.sync.dma_start(out_p[qt * P : (qt + 1) * P, ng * 512 : (ng + 1) * 512], o[:])

    # emission order: early V tiles + first pair's QK, then attention with
    # next pair's QK units interleaved as PE filler
    for rt in range(4):
        emit_v(rt)
    emit_qk(0)
    emit_qk(DL // P)
    for rt in range(4, NRT):
        emit_v(rt)
    for pair in range(HPC // 2):
        if pair + 1 < HPC // 2:
            fill = [
                (lambda mt=mt, rg=rg: qk_unit(mt, rg))
                for mt in (pair + 1, DL // P + pair + 1)
                for rg in range(NQG)
            ]
        else:
            fill = []
        emit_pair(pair, fill)
    if DEBUG:
        nc.sync.dma_start(dbg["dbg_qkvT"], qkvT[:].rearrange("p a b -> p (a b)"))
        nc.sync.dma_start(dbg["dbg_vsb"], vsb[:].rearrange("p a b c -> p (a b c)"))
        nc.sync.dma_start(dbg["dbg_yT"], yT[:].rearrange("p a b -> p (a b)"))
    for qt in range(NRT):
        emit_proj(qt)


def _build():
    if "nc" in _CACHE:
        return _CACHE["nc"]
    nc = bacc.Bacc("TRN2", target_bir_lowering=False, debug=False, num_devices=NCORES)
    xt = nc.dram_tensor("xt", [D, S], MM_DT, kind="ExternalInput").ap()
    wqkv = nc.dram_tensor("wqkv", [D, 3 * DL], MM_DT, kind="ExternalInput").ap()
    wproj = nc.dram_tensor("wproj", [DL, D], MM_DT, kind="ExternalInput").ap()
    bqk = nc.dram_tensor("bqk", [P, NMT], dt.float32, kind="ExternalInput").ap()
    bv = nc.dram_tensor("bv", [1, DL], dt.float32r, kind="ExternalInput").ap()
    bp = nc.dram_tensor("bp", [1, D], dt.float32r, kind="ExternalInput").ap()
    maskd = nc.dram_tensor("maskd", [P, 4, 512], MM_DT, kind="ExternalInput").ap()
    ones32 = nc.dram_tensor("ones32", [1, P], dt.float32, kind="ExternalInput").ap()
    onesr = nc.dram_tensor("onesr", [1, P], dt.float32r, kind="ExternalInput").ap()
    out_p = nc.dram_tensor("out_p", [S, D], dt.float32, kind="ExternalOutput").ap()

    dbg = {}
    if DEBUG:
        dbg["dbg_qkvT"] = nc.dram_tensor("dbg_qkvT", [P, NMT * S], MM_DT, kind="ExternalOutput").ap()
        dbg["dbg_vsb"] = nc.dram_tensor("dbg_vsb", [P, NKT * HPC * (HD + 1)], MM_DT, kind="ExternalOutput").ap()
        dbg["dbg_yT"] = nc.dram_tensor("dbg_yT", [P, (DL // P) * S], MM_DT, kind="ExternalOutput").ap()
        dbg["dbg_pt"] = nc.dram_tensor("dbg_pt", [P, GK * 512], dt.float32, kind="ExternalOutput").ap()
        dbg["dbg_av"] = nc.dram_tensor("dbg_av", [P, 512], dt.float32, kind="ExternalOutput").ap()

    io = (xt, wqkv, wproj, bqk, bv, bp, maskd, ones32, onesr, out_p)
    with tile.TileContext(nc) as tc, ExitStack() as ctx:
        _emit(nc, tc, ctx, io, dbg)
    nc.compile()
    _CACHE["nc"] = nc
    return nc


def _in_maps(x, w_qkv, b_qkv, w_proj, b_proj):
    x = np.asarray(x, dtype=np.float32)
    w_qkv = np.asarray(w_qkv, dtype=np.float32)
    b_qkv = np.asarray(b_qkv, dtype=np.float32)
    w_proj = np.asarray(w_proj, dtype=np.float32)
    b_proj = np.asarray(b_proj, dtype=np.float32)

    # causal mask for the 4 diagonal-tile alignments: [128, 4, 512]
    kp = np.arange(P)[:, None, None]
    td = np.arange(4)[None, :, None]
    qf = np.arange(512)[None, None, :]
    maskd = ((P * td + kp) <= qf).astype(NP_MM)
    ones = np.ones((1, P), dtype=np.float32)

    maps = []
    for c in range(NCORES):
        b, half = divmod(c, 2)
        lo, hi = half * DL, (half + 1) * DL
        wq = w_qkv[:, lo:hi]
        wk = w_qkv[:, D + lo : D + hi]
        wv = w_qkv[:, 2 * D + lo : 2 * D + hi]
        wqkv_l = np.concatenate([wq, wk, wv], axis=1).astype(NP_MM)
        bqk_l = np.concatenate([b_qkv[lo:hi], b_qkv[D + lo : D + hi]])
        bqk_t = np.ascontiguousarray(bqk_l.reshape(NMT, P).T)  # [128, 8]
        maps.append(
            {
                "xt": np.ascontiguousarray(x[b].T).astype(NP_MM),
                "wqkv": wqkv_l,
                "wproj": w_proj[lo:hi, :].astype(NP_MM),
                "bqk": bqk_t,
                "bv": b_qkv[2 * D + lo : 2 * D + hi].reshape(1, DL),
                "bp": (0.5 * b_proj).reshape(1, D),
                "maskd": maskd,
                "ones32": ones,
                "onesr": ones,
            }
        )
    return maps


def _run(x, w_qkv, b_qkv, w_proj, b_proj, trace=False):
    nc = _build()
    maps = _in_maps(x, w_qkv, b_qkv, w_proj, b_proj)
    res = run_bass_kernel_spmd(nc, maps, list(range(NCORES)), trace=trace)
    out = np.empty((B, S, D), dtype=np.float32)
    for b in range(B):
        out[b] = res.results[2 * b]["out_p"] + res.results[2 * b + 1]["out_p"]
    return out, res


def kernel(x, w_qkv, b_qkv, w_proj, b_proj):
    out, _ = _run(x, w_qkv, b_qkv, w_proj, b_proj)
    return out


# revision 12
# speedup vs baseline: 1.2869x; 1.0239x over previous
"""Causal self-attention (B=4, S=2048, D=1024, H=16, fp32) on 8 TRN2 NeuronCores.

Sharding (hybrid batch x heads): core c handles batch b = c//2 and head half
h = c%2 (8 heads, 512 channels). Each core computes:
  qkv^T slice  = (x[b] @ w_qkv[:, local])^T      via PE, transposed formulation
  per head: scores^T = K Q^T / 8, causal exp (no max subtraction -- scores are
  O(1) by construction), AV with ones-augmented V giving row sums for free,
  y^T = AV^T * (1/rowsum) broadcast via K=1 ones-matmul,
  partial out = y @ w_proj[local rows] + 0.5 * b_proj.
Host sums the two half partials per batch and stacks.

Matmul dtype is switchable: bfloat16 (fast) or float32r (tf32-class accuracy).
"""

import numpy as np
import ml_dtypes
from contextlib import ExitStack

import concourse.bass as bass
import concourse.mybir as mybir
import concourse.tile as tile
from concourse import bacc
from concourse.bass_utils import run_bass_kernel_spmd

dt = mybir.dt
AF = mybir.ActivationFunctionType

B, S, D, H, HD = 4, 2048, 1024, 16, 64
NCORES = 8
HPC = 8            # heads per core
DL = HPC * HD      # local channel width (512)
P = 128
NQG = S // 512     # q groups of 512          -> 4
NKT = S // P       # k tiles of 128           -> 16
NDC = D // P       # D chunks of 128          -> 8
NMT = 2 * DL // P  # q^T+k^T m-tiles           -> 8
NRT = S // P       # row tiles for V / proj   -> 16
GK = 1             # k-tiles per exp group

MM_DT = dt.bfloat16
NP_MM = ml_dtypes.bfloat16

DEBUG = False          # extra per-stage debug outputs (core-0 comparison)
RECIP_FAST = False     # use reciprocal_approx_fast instead of reciprocal
MASK_GPSIMD = False    # apply causal mask on GpSimd instead of VectorE

_CACHE = {}


def _emit(nc, tc, ctx, io, dbg):
    xt, wqkv, wproj, bqk, bv, bp, maskd, ones32, onesr, out_p = io

    const = ctx.enter_context(tc.tile_pool(name="const", bufs=1))
    big = ctx.enter_context(tc.tile_pool(name="big", bufs=1))
    ptp = ctx.enter_context(tc.tile_pool(name="ptp", bufs=8))
    ep = ctx.enter_context(tc.tile_pool(name="ep", bufs=3))
    outp = ctx.enter_context(tc.tile_pool(name="outp", bufs=3))
    bigps = ctx.enter_context(tc.tile_pool(name="bigps", bufs=4, space="PSUM"))
    mmps = ctx.enter_context(tc.tile_pool(name="mmps", bufs=2, space="PSUM"))
    avps = ctx.enter_context(tc.tile_pool(name="avps", bufs=2, space="PSUM"))

    # ---- constants / weights ----
    bqk_sb = const.tile([P, NMT], dt.float32, tag="bqk")
    nc.sync.dma_start(bqk_sb[:], bqk)
    bv_sb = const.tile([1, DL], dt.float32r, tag="bv")
    nc.sync.dma_start(bv_sb[:], bv)
    bp_sb = const.tile([1, D], dt.float32r, tag="bp")
    nc.sync.dma_start(bp_sb[:], bp)
    mask_sb = const.tile([P, 4, 512], MM_DT, tag="mask")
    nc.sync.dma_start(mask_sb[:], maskd)
    ones32_sb = const.tile([1, P], dt.float32, tag="ones32")
    nc.sync.dma_start(ones32_sb[:], ones32)
    onesr_sb = const.tile([1, P], dt.float32r, tag="onesr")
    nc.sync.dma_start(onesr_sb[:], onesr)
    ones512_sb = const.tile([HD + 1, 512], dt.float32, tag="ones512")
    nc.vector.memset(ones512_sb[:], 1.0)

    xt_sb = []
    for c in range(NDC):
        t = const.tile([P, S], MM_DT, tag=f"xt{c}")
        nc.sync.dma_start(t[:], xt[c * P : (c + 1) * P, :])
        xt_sb.append(t)
    wq_sb = []
    for c in range(NDC):
        t = const.tile([P, 3 * DL], MM_DT, tag=f"wq{c}")
        nc.sync.dma_start(t[:], wqkv[c * P : (c + 1) * P, :])
        wq_sb.append(t)
    wp_sb = []
    for c in range(DL // P):
        t = const.tile([P, D], MM_DT, tag=f"wp{c}")
        nc.sync.dma_start(t[:], wproj[c * P : (c + 1) * P, :])
        wp_sb.append(t)

    # ---- persistent intermediates ----
    qkvT = const.tile([P, NMT, S], MM_DT, tag="qkvT")     # q^T then k^T
    vsb = const.tile([P, NKT, HPC, HD + 1], MM_DT, tag="vsb")
    yT = const.tile([P, DL // P, S], MM_DT, tag="yT")

    nc.gpsimd.memset(vsb[:], 1.0)  # ones column at [..., HD]

    # ---- phase 1b: V in natural layout [row, (h, hd)] ----
    def emit_v(rt):
        ps = mmps.tile([P, DL], dt.float32, tag="mm")
        for c in range(NDC):
            nc.tensor.matmul(
                ps[:],
                xt_sb[c][:, rt * P : (rt + 1) * P],
                wq_sb[c][:, 2 * DL : 3 * DL],
                start=(c == 0),
                stop=False,
            )
        nc.tensor.matmul(
            ps[:], onesr_sb[:], bv_sb[:], start=False, stop=True
        )
        nc.vector.tensor_copy(
            vsb[:, rt, :, 0:HD],
            ps[:].rearrange("p (h d) -> p h d", d=HD),
        )

    # ---- phase 1a: q^T / k^T m-tiles ----
    def qk_unit(mt, rg):
        ps = mmps.tile([P, 512], dt.float32, tag="mm", name=f"qk{mt}_{rg}")
        for c in range(NDC):
            nc.tensor.matmul(
                ps[:],
                wq_sb[c][:, mt * P : (mt + 1) * P],
                xt_sb[c][:, rg * 512 : (rg + 1) * 512],
                start=(c == 0),
                stop=(c == NDC - 1),
            )
        nc.scalar.activation(
            qkvT[:, mt, rg * 512 : (rg + 1) * 512],
            ps[:],
            AF.Identity,
            bias=bqk_sb[:, mt : mt + 1],
        )

    def emit_qk(mt):
        for rg in range(NQG):
            qk_unit(mt, rg)

    # ---- phase 2: attention for a head pair (concurrent K=64 row groups) ----
    def emit_pair(pr, fillers=()):
        hs = (2 * pr, 2 * pr + 1)
        fillers = list(fillers)
        fill_i = 0

        def epi(h, av, qg):
            pb = (h % 2) * 64
            if DEBUG and h == 0 and qg == 0:
                dav = ep.tile([P, 512], dt.float32, tag="dav")
                nc.vector.tensor_copy(dav[:], av[:])
                nc.sync.dma_start(dbg["dbg_av"], dav[:])
            yraw = ep.tile([HD + 1, 512], dt.float32, tag="yraw")
            nc.vector.tensor_copy(yraw[:], av[0 : HD + 1, :])
            rcp = ep.tile([1, 512], dt.float32r, tag="rcp")
            with nc.allow_low_precision(reason="softmax denominators ~1e3; f32r err ~1e-4"):
                nc.vector.reciprocal(rcp[:], yraw[HD : HD + 1, :])
            bc = mmps.tile([64, 512], dt.float32, tag="mm", name="bc")
            nc.tensor.matmul(bc[:], onesr_sb[:, 0:64], rcp[:], start=True, stop=True)
            nc.vector.tensor_mul(
                yT[pb : pb + 64, h // 2, qg * 512 : (qg + 1) * 512],
                yraw[0:HD, :],
                bc[:],
            )

        LAG = 2
        nfill_slots = sum(4 * qg + 4 for qg in range(NQG))
        fill_every = max(1, nfill_slots // max(1, len(fillers))) if fillers else 0
        slot = 0
        for qg in range(NQG):
            nkt = 4 * qg + 4
            av = {h: avps.tile([P, 512], dt.float32, tag="av", name=f"av{h}") for h in hs}
            pts = {}
            for step in range(nkt + LAG):
                if step < nkt:
                    kt = step
                    sps = {h: bigps.tile([P, 512], dt.float32, tag="big", name=f"sps{h}") for h in hs}
                    for h in hs:
                        pb = (h % 2) * 64
                        nc.tensor.matmul(
                            sps[h][:],
                            qkvT[pb : pb + 64, DL // P + h // 2, kt * P : (kt + 1) * P],
                            qkvT[pb : pb + 64, h // 2, qg * 512 : (qg + 1) * 512],
                            start=True,
                            stop=True,
                        )
                    td = kt - 4 * qg
                    for h in hs:
                        pth = ptp.tile([P, 512], MM_DT, tag="pt", name=f"pt{h}")
                        nc.scalar.activation(pth[:], sps[h][:], AF.Exp, scale=0.125)
                        if td >= 0:
                            eng = nc.gpsimd if MASK_GPSIMD else nc.vector
                            eng.tensor_tensor(
                                pth[:], pth[:], mask_sb[:, td, :], mybir.AluOpType.mult
                            )
                        pts[(h, kt)] = pth
                if step >= LAG:
                    kt = step - LAG
                    for h in hs:
                        nc.tensor.matmul(
                            av[h][0 : HD + 1, :],
                            vsb[:, kt, h, :],
                            pts.pop((h, kt))[:],
                            start=(kt == 0),
                            stop=(kt == nkt - 1),
                        )
                slot += 1
                if fillers and fill_i < len(fillers) and slot % fill_every == 0:
                    fillers[fill_i]()
                    fill_i += 1
            for h in hs:
                epi(h, av[h], qg)
        while fill_i < len(fillers):
            fillers[fill_i]()
            fill_i += 1

    # ---- phase 3: projection ----
    def emit_proj(qt):
        for ng in range(D // 512):
            ps = mmps.tile([P, 512], dt.float32, tag="mm")
            for c in range(DL // P):
                nc.tensor.matmul(
                    ps[:],
                    yT[:, c, qt * P : (qt + 1) * P],
                    wp_sb[c][:, ng * 512 : (ng + 1) * 512],
                    start=(c == 0),
                    stop=False,
                )
            nc.tensor.matmul(
                ps[:], onesr_sb[:], bp_sb[:, ng * 512 : (ng + 1) * 512],
                start=False, stop=True,
            )
            o = outp.tile([P, 512], dt.float32, tag="o")
            nc.scalar.activation(o[:], ps[:], AF.Copy)
            nc.sync.dma_start(out_p[qt * P : (qt + 1) * P, ng * 512 : (ng + 1) * 512], o[:])

    # emission order: early V tiles + first pair's QK, then attention with
    # next pair's QK units interleaved as PE filler
    for rt in range(4):
        emit_v(rt)
    emit_qk(0)
    emit_qk(DL // P)
    for rt in range(4, NRT):
        emit_v(rt)
    for pair in range(HPC // 2):
        if pair + 1 < HPC // 2:
            fill = [
                (lambda mt=mt, rg=rg: qk_unit(mt, rg))
                for mt in (pair + 1, DL // P + pair + 1)
                for rg in range(NQG)
            ]
        else:
            fill = []
        emit_pair(pair, fill)
    if DEBUG:
        nc.sync.dma_start(dbg["dbg_qkvT"], qkvT[:].rearrange("p a b -> p (a b)"))
        nc.sync.dma_start(dbg["dbg_vsb"], vsb[:].rearrange("p a b c -> p (a b c)"))
        nc.sync.dma_start(dbg["dbg_yT"], yT[:].rearrange("p a b -> p (a b)"))
    for qt in range(NRT):
        emit_proj(qt)


def _build():
    if "nc" in _CACHE:
        return _CACHE["nc"]
    nc = bacc.Bacc("TRN2", target_bir_lowering=False, debug=False, num_devices=NCORES)
    xt = nc.dram_tensor("xt", [D, S], MM_DT, kind="ExternalInput").ap()
    wqkv = nc.dram_tensor("wqkv", [D, 3 * DL], MM_DT, kind="ExternalInput").ap()
    wproj = nc.dram_tensor("wproj", [DL, D], MM_DT, kind="ExternalInput").ap()
    bqk = nc.dram_tensor("bqk", [P, NMT], dt.float32, kind="ExternalInput").ap()
    bv = nc.dram_tensor("bv", [1, DL], dt.float32r, kind="ExternalInput").ap()
    bp = nc.dram_tensor("bp", [1, D], dt.float32r, kind="ExternalInput").ap()
    maskd = nc.dram_tensor("maskd", [P, 4, 512], MM_DT, kind="ExternalInput").ap()
    ones32 = nc.dram_tensor("ones32", [1, P], dt.float32, kind="ExternalInput").ap()
    onesr = nc.dram_tensor("onesr", [1, P], dt.float32r, kind="ExternalInput").ap()
    out_p = nc.dram_tensor("out_p", [S, D], dt.float32, kind="ExternalOutput").ap()

    dbg = {}
    if DEBUG:
        dbg["dbg_qkvT"] = nc.dram_tensor("dbg_qkvT", [P, NMT * S], MM_DT, kind="ExternalOutput").ap()
        dbg["dbg_vsb"] = nc.dram_tensor("dbg_vsb", [P, NKT * HPC * (HD + 1)], MM_DT, kind="ExternalOutput").ap()
        dbg["dbg_yT"] = nc.dram_tensor("dbg_yT", [P, (DL // P) * S], MM_DT, kind="ExternalOutput").ap()
        dbg["dbg_pt"] = nc.dram_tensor("dbg_pt", [P, GK * 512], dt.float32, kind="ExternalOutput").ap()
        dbg["dbg_av"] = nc.dram_tensor("dbg_av", [P, 512], dt.float32, kind="ExternalOutput").ap()

    io = (xt, wqkv, wproj, bqk, bv, bp, maskd, ones32, onesr, out_p)
    with tile.TileContext(nc) as tc, ExitStack() as ctx:
        _emit(nc, tc, ctx, io, dbg)
    nc.compile()
    _CACHE["nc"] = nc
    return nc


def _in_maps(x, w_qkv, b_qkv, w_proj, b_proj):
    x = np.asarray(x, dtype=np.float32)
    w_qkv = np.asarray(w_qkv, dtype=np.float32)
    b_qkv = np.asarray(b_qkv, dtype=np.float32)
    w_proj = np.asarray(w_proj, dtype=np.float32)
    b_proj = np.asarray(b_proj, dtype=np.float32)

    # causal mask for the 4 diagonal-tile alignments: [128, 4, 512]
    kp = np.arange(P)[:, None, None]
    td = np.arange(4)[None, :, None]
    qf = np.arange(512)[None, None, :]
    maskd = ((P * td + kp) <= qf).astype(NP_MM)
    ones = np.ones((1, P), dtype=np.float32)

    maps = []
    for c in range(NCORES):
        b, half = divmod(c, 2)
        lo, hi = half * DL, (half + 1) * DL
        wq = w_qkv[:, lo:hi]
        wk = w_qkv[:, D + lo : D + hi]
        wv = w_qkv[:, 2 * D + lo : 2 * D + hi]
        wqkv_l = np.concatenate([wq, wk, wv], axis=1).astype(NP_MM)
        bqk_l = np.concatenate([b_qkv[lo:hi], b_qkv[D + lo : D + hi]])
        bqk_t = np.ascontiguousarray(bqk_l.reshape(NMT, P).T)  # [128, 8]
        maps.append(
            {
                "xt": np.ascontiguousarray(x[b].T).astype(NP_MM),
                "wqkv": wqkv_l,
                "wproj": w_proj[lo:hi, :].astype(NP_MM),
                "bqk": bqk_t,
                "bv": b_qkv[2 * D + lo : 2 * D + hi].reshape(1, DL),
                "bp": (0.5 * b_proj).reshape(1, D),
                "maskd": maskd,
                "ones32": ones,
                "onesr": ones,
            }
        )
    return maps


def _run(x, w_qkv, b_qkv, w_proj, b_proj, trace=False):
    nc = _build()
    maps = _in_maps(x, w_qkv, b_qkv, w_proj, b_proj)
    res = run_bass_kernel_spmd(nc, maps, list(range(NCORES)), trace=trace)
    out = np.empty((B, S, D), dtype=np.float32)
    for b in range(B):
        out[b] = res.results[2 * b]["out_p"] + res.results[2 * b + 1]["out_p"]
    return out, res


def kernel(x, w_qkv, b_qkv, w_proj, b_proj):
    out, _ = _run(x, w_qkv, b_qkv, w_proj, b_proj)
    return out
